# revision 2
# baseline (speedup 1.0000x reference)
"""Trainium2 Bass kernel for nn_DiscretePredictor (gnn_message_passing), v2.

Pair-major rewrite of the big phase.  Reference computation per batch b
(256 global, n=32 objects, d=128):
    x[(b,i,j)] = a1*(u_i + v_j) + z0   (u = state@W1a.T, v = state@W1b.T,
                                        BN1 folded: a1, z0 from global stats)
    msum[f,(b,i)] = sum_j e_ij*LeakyReLU(x) + (32-deg_i)*c0,  c0 = LR(z0)
then agg = W2@msum + 32*b2 and a second MLP (BN2, LeakyReLU) as in v1.

Key idea vs v1: put PAIRS on partitions and FEATURES on the free dim for
the big phase.  Then every expensive step is a PE matmul (cost = out free
size only, f32r = 1 cyc/row at N>=256):
  - x-tile [128 pairs, 256 f] = sel_(b%2,ib) @ UVALL[:,q,:]   (1 matmul)
    where UVALL packs u'=a1*u+z0/2, v'=a1*v+z0/2 rows per batch pair
  - LeakyReLU split: LR(x) = 0.99*relu(x) + 0.01*x.  relu is the only
    elementwise pass (PSUM->SBUF), split across ACT and DVE.
  - masked j-reduction: matmul with 0.99*e as a [128,4] rhs -> 4 final
    feature-major msumT columns per tile (essentially free, N=4)
  - the 0.01*x linear part is pure PE algebra straight into the same
    PSUM regions: statedeg@w1as (0.01*deg*a1u), K=2 rank-2 init
    (c0*(32-deg) + z0/2*0.01*deg), and UVALL@etbq01 (0.01*E(a1v+z0/2)).
BN stats (both layers) need global sums -> two [128,4] AllReduces.
Stats for BN1 are computed from unscaled row-major u/v via PE matmuls
against degree columns plus the same M = sum_b S_b^T E_b S_b cross-term
trick as v1.

Sharding: data-parallel over batch (32 batches per core), params replicated.
"""

import os
import sys

for p in ("/opt/trn_rl_repo", "/root/.axon_site", "/root/.axon_site/_ro/trn_rl_repo",
          "/root/.axon_site/_ro/pypackages"):
    if os.path.isdir(p) and p not in sys.path:
        sys.path.append(p)

import numpy as np

import concourse.bass as bass
import concourse.mybir as mybir
import concourse.tile as tile
from concourse import bacc
from concourse.bass_utils import run_bass_kernel_spmd

F32 = mybir.dt.float32
F32R = mybir.dt.float32r
AF = mybir.ActivationFunctionType
ALU = mybir.AluOpType

B = 256          # global batch
NOBJ = 32        # objects per batch
D = 128          # object dim
F = 256          # hidden width (both MLPs)
NCORES = 8
NB = B // NCORES          # batches per core = 32
ROWS = NB * NOBJ          # (b,i) rows per core = 1024
NT = NB * (NOBJ // 4)     # pair-tiles per core = 256 (4 i x 32 j each)
N1 = float(B * NOBJ * NOBJ)   # BN1 row count (global) = 262144
N2 = float(B * NOBJ)          # BN2 row count (global) = 8192
EPS = 1e-5
SLOPE = 0.01
NO_CC = os.environ.get("BASS_NO_CC", "0") == "1"
STAGE = int(os.environ.get("BASS_STAGE", "9"))
DVE_RELU = os.environ.get("BASS_DVE_RELU", "1") == "1"  # split relu ACT/DVE


def _build_nc():
    nc = bacc.Bacc("TRN2", target_bir_lowering=False, debug=False,
                   enable_asserts=True, num_devices=NCORES)

    # ---- per-core device I/O (f32r tensors receive plain f32 bytes) ----
    stateT_d = nc.dram_tensor("stateT", [D, ROWS], F32R, kind="ExternalInput")
    statedeg_d = nc.dram_tensor("statedeg", [D, ROWS], F32R, kind="ExternalInput")
    stateTu_d = nc.dram_tensor("stateTu", [D, 16 * D], F32R, kind="ExternalInput")
    stateTv_d = nc.dram_tensor("stateTv", [D, 16 * D], F32R, kind="ExternalInput")
    staterm_d = nc.dram_tensor("state_rm", [ROWS, D], F32R, kind="ExternalInput")
    etbd_d = nc.dram_tensor("etbd_all", [D, 8 * D], F32R, kind="ExternalInput")
    sel_d = nc.dram_tensor("sel_all", [D, 16 * D], F32R, kind="ExternalInput")
    eblk_d = nc.dram_tensor("eblk_all", [D, ROWS], F32R, kind="ExternalInput")
    etbq_d = nc.dram_tensor("etbq_all", [D, 16 * 64], F32R, kind="ExternalInput")
    degA_d = nc.dram_tensor("degA", [2, ROWS], F32R, kind="ExternalInput")
    degcol_d = nc.dram_tensor("degcol", [D, 16], F32R, kind="ExternalInput")
    cdegcol_d = nc.dram_tensor("cdegcol", [D, 16], F32R, kind="ExternalInput")
    w1aT_d = nc.dram_tensor("w1aT", [D, F], F32R, kind="ExternalInput")
    w1bT_d = nc.dram_tensor("w1bT", [D, F], F32R, kind="ExternalInput")
    w2T_d = nc.dram_tensor("w2T", [F, D], F32R, kind="ExternalInput")
    fw1T_d = nc.dram_tensor("fw1T", [2 * D, F], F32R, kind="ExternalInput")
    fw2T_d = nc.dram_tensor("fw2T", [F, D], F32R, kind="ExternalInput")
    g1_d = nc.dram_tensor("g1", [F], F32, kind="ExternalInput")
    be1_d = nc.dram_tensor("be1", [F], F32, kind="ExternalInput")
    b2_d = nc.dram_tensor("b2", [D], F32, kind="ExternalInput")
    g2_d = nc.dram_tensor("g2", [F], F32, kind="ExternalInput")
    be2_d = nc.dram_tensor("be2", [F], F32, kind="ExternalInput")
    fb2_d = nc.dram_tensor("fb2", [D], F32, kind="ExternalInput")
    outT_d = nc.dram_tensor("outT", [D, ROWS], F32, kind="ExternalOutput")

    from contextlib import ExitStack
    with tile.TileContext(nc) as tc, ExitStack() as ctx:
        consts = ctx.enter_context(tc.tile_pool(name="consts", bufs=1))
        uvp = ctx.enter_context(tc.tile_pool(name="uv", bufs=1))
        big = ctx.enter_context(tc.tile_pool(name="big", bufs=2))
        statp = ctx.enter_context(tc.tile_pool(name="stats", bufs=1))
        psum = ctx.enter_context(tc.tile_pool(name="psum", bufs=1, space="PSUM"))
        dram = ctx.enter_context(tc.tile_pool(name="dram", bufs=1, space="DRAM"))

        # ---------------- setup: load consts ----------------
        sT = consts.tile([D, ROWS], F32R)
        nc.sync.dma_start(out=sT[:], in_=stateT_d.ap())
        sdeg = consts.tile([D, ROWS], F32R)
        nc.sync.dma_start(out=sdeg[:], in_=statedeg_d.ap())
        sTu = consts.tile([D, 16, D], F32R)
        nc.sync.dma_start(out=sTu[:], in_=stateTu_d.ap().rearrange("p (s k) -> p s k", s=16))
        sTv = consts.tile([D, 16, D], F32R)
        nc.sync.dma_start(out=sTv[:], in_=stateTv_d.ap().rearrange("p (s k) -> p s k", s=16))
        selc = consts.tile([D, 16, D], F32R)
        nc.sync.dma_start(out=selc[:], in_=sel_d.ap().rearrange("p (s k) -> p s k", s=16))
        eblk = consts.tile([D, ROWS], F32R)
        nc.sync.dma_start(out=eblk[:], in_=eblk_d.ap())
        etbq = consts.tile([D, 16, 64], F32R)
        nc.sync.dma_start(out=etbq[:], in_=etbq_d.ap().rearrange("p (s k) -> p s k", s=16))
        etbbd = consts.tile([D, 8, D], F32R)
        nc.sync.dma_start(out=etbbd[:], in_=etbd_d.ap().rearrange("p (s k) -> p s k", s=8))
        degA = consts.tile([2, ROWS], F32R)
        nc.sync.dma_start(out=degA[:], in_=degA_d.ap())
        degcol = consts.tile([D, 16], F32R)
        nc.sync.dma_start(out=degcol[:], in_=degcol_d.ap())
        cdegcol = consts.tile([D, 16], F32R)
        nc.sync.dma_start(out=cdegcol[:], in_=cdegcol_d.ap())
        w1a = consts.tile([D, F], F32R)
        w1b = consts.tile([D, F], F32R)
        nc.sync.dma_start(out=w1a[:], in_=w1aT_d.ap())
        nc.sync.dma_start(out=w1b[:], in_=w1bT_d.ap())
        w2k = consts.tile([D, 2, D], F32R)
        fw1 = consts.tile([D, 2, F], F32R)
        fw2 = consts.tile([D, 2, D], F32R)
        nc.sync.dma_start(out=w2k[:], in_=w2T_d.ap().rearrange("(k p) d -> p k d", p=D))
        nc.sync.dma_start(out=fw1[:], in_=fw1T_d.ap().rearrange("(k p) f -> p k f", p=D))
        nc.sync.dma_start(out=fw2[:], in_=fw2T_d.ap().rearrange("(k p) d -> p k d", p=D))

        def fvec(dh, nm):  # [256] dram vector -> [128, 2] feature-major sbuf
            t = consts.tile([D, 2], F32, tag=nm, name=nm)
            nc.sync.dma_start(out=t[:], in_=dh.ap().rearrange("(h p) -> p h", p=D))
            return t

        def dvec(dh, nm):  # [128] -> [128, 1]
            t = consts.tile([D, 1], F32, tag=nm, name=nm)
            nc.sync.dma_start(out=t[:], in_=dh.ap().rearrange("(h p) -> p h", p=D))
            return t

        g1c, be1c = fvec(g1_d, "g1c"), fvec(be1_d, "be1c")
        g2c, be2c = fvec(g2_d, "g2c"), fvec(be2_d, "be2c")
        b2c, fb2c = dvec(b2_d, "b2c"), dvec(fb2_d, "fb2c")
        b2x32 = consts.tile([D, 1], F32)
        nc.vector.tensor_scalar_mul(b2x32[:], b2c[:], float(NOBJ))

        # ---------------- stats: row-major u/v + degree matmuls -------------
        # u_rm/v_rm [128 rows, 8 rt, 256 f] unscaled
        u_rm = uvp.tile([D, 8, F], F32R, tag="u_rm", name="u_rm")
        v_rm = uvp.tile([D, 8, F], F32R, tag="v_rm", name="v_rm")
        statps = psum.tile([D, 128], F32, tag="red0", name="statps", bufs=1)
        for r in range(8):
            for uv, (dst, w) in enumerate(((u_rm, w1a), (v_rm, w1b))):
                ps = psum.tile([D, F], F32, tag="span", bufs=2)
                nc.tensor.matmul(ps[:], sT[:, r * D:(r + 1) * D], w[:],
                                 start=True, stop=True)
                if (r + uv) % 2 == 0:
                    nc.scalar.activation(out=dst[:, r, :], in_=ps[:], func=AF.Copy)
                else:
                    nc.vector.tensor_copy(dst[:, r, :], ps[:])
        # squares -> u2/v2 rotating; deg-weighted sums into statps columns:
        # cols 0,1: sum deg*u (fh0, fh1); 2,3: sum cdeg*v; 4,5: deg*u^2; 6,7: cdeg*v^2
        for r in range(8):
            for uv, (src, dcol) in enumerate(((u_rm, degcol), (v_rm, cdegcol))):
                sq = big.tile([D, F], F32R, tag="sq", bufs=3, name=f"sq{r}{uv}")
                if (r + uv) % 2 == 0:
                    nc.scalar.activation(out=sq[:], in_=src[:, r, :], func=AF.Square)
                else:
                    nc.vector.tensor_mul(sq[:], src[:, r, :], src[:, r, :])
                for fh in range(2):
                    c = 2 * uv + fh
                    nc.tensor.matmul(statps[:, 2 * (8 * c + r):2 * (8 * c + r) + 2],
                                     src[:, r, fh * D:(fh + 1) * D],
                                     dcol[:, 2 * r:2 * r + 2],
                                     start=True, stop=True)
                    nc.tensor.matmul(statps[:, 64 + 2 * (8 * c + r):66 + 2 * (8 * c + r)],
                                     sq[:, fh * D:(fh + 1) * D],
                                     dcol[:, 2 * r:2 * r + 2],
                                     start=True, stop=True)

        # ---------------- cross term: 2 * w1a_f^T M w1b_f -------------------
        onesf = statp.tile([D, D], F32)
        nc.vector.memset(onesf[:], 1.0)
        ones_mat = statp.tile([D, D], F32R)
        nc.vector.tensor_copy(ones_mat[:], onesf[:])
        mps = psum.tile([D, D], F32, tag="red1", bufs=1, name="mps")
        for g in range(8):
            spack = big.tile([D, D], F32R, tag="spack", bufs=3, name=f"spack{g}")
            nc.sync.dma_start(out=spack[:], in_=staterm_d.ap()[g * D:(g + 1) * D, :])
            esg = psum.tile([D, D], F32, tag="span", bufs=2, name=f"esg{g}")
            nc.tensor.matmul(esg[:], etbbd[:, g, :], spack[:], start=True, stop=True)
            esgs = big.tile([D, D], F32R, tag="esgs", bufs=3, name=f"esgs{g}")
            nc.scalar.activation(out=esgs[:], in_=esg[:], func=AF.Copy)
            nc.tensor.matmul(mps[:], spack[:], esgs[:],
                             start=(g == 0), stop=(g == 7))
        msb = statp.tile([D, D], F32R)
        nc.scalar.activation(out=msb[:], in_=mps[:], func=AF.Copy)
        t1p = psum.tile([D, F], F32, tag="span", bufs=2)
        nc.tensor.matmul(t1p[:], msb[:], w1a[:], start=True, stop=True)
        t2 = statp.tile([D, F], F32R)
        nc.vector.tensor_mul(t2[:], t1p[:], w1b[:])
        crp = psum.tile([D, F], F32, tag="span", bufs=2)
        nc.tensor.matmul(crp[:], ones_mat[:], t2[:], start=True, stop=True)
        crsb = statp.tile([1, F], F32)
        nc.scalar.activation(out=crsb[:], in_=crp[0:1, :], func=AF.Copy)
        crd = dram.tile([1, F], F32, tag="crd", name="crd")
        nc.sync.dma_start(out=crd[:], in_=crsb[:])
        crossc = statp.tile([D, 2], F32)
        nc.sync.dma_start(out=crossc[:],
                          in_=crd[:].rearrange("x (h p) -> (x p) h", p=D))

        # ---------------- BN1 stats allreduce + coeffs ----------------
        shpart = statp.tile([D, 8], F32)
        nc.vector.reduce_sum(
            shpart[:],
            statps[:].rearrange("p (c r k) -> p c (r k)", c=8, k=2),
            axis=mybir.AxisListType.X)
        stat1 = statp.tile([D, 4], F32)
        nc.vector.tensor_add(stat1[:, 0:2], shpart[:, 0:2], shpart[:, 2:4])
        sqsum = statp.tile([D, 2], F32)
        nc.vector.tensor_add(sqsum[:], shpart[:, 4:6], shpart[:, 6:8])
        cr2 = statp.tile([D, 2], F32)
        nc.vector.tensor_scalar_mul(cr2[:], crossc[:], 2.0)
        nc.vector.tensor_add(stat1[:, 2:4], sqsum[:], cr2[:])
        cc1_in = dram.tile([D, 4], F32, tag="cc1i")
        cc1_out = dram.tile([D, 4], F32, tag="cc1o")
        nc.sync.dma_start(out=cc1_in[:], in_=stat1[:])
        if NO_CC:
            nc.sync.dma_start(out=cc1_out[:], in_=cc1_in[:])
        else:
            nc.gpsimd.collective_compute(
                "AllReduce", ALU.add, replica_groups=[list(range(NCORES))],
                ins=[cc1_in[:].opt()], outs=[cc1_out[:].opt()])
        statg1 = statp.tile([D, 4], F32)
        nc.sync.dma_start(out=statg1[:], in_=cc1_out[:])
        epsc = statp.tile([D, 1], F32)
        nc.vector.memset(epsc[:], EPS)

        def bn_coeffs(statg, n_rows, gc, bec, pool, pre):
            def tl(nm):
                return pool.tile([D, 2], F32, tag=pre + nm, name=pre + nm)
            mean, msq, var = tl("mean"), tl("msq"), tl("var")
            sd, rstd, a, ma, z = tl("sd"), tl("rstd"), tl("a"), tl("ma"), tl("z")
            nc.scalar.activation(out=mean[:], in_=statg[:, 0:2], func=AF.Copy,
                                 scale=1.0 / n_rows)
            nc.scalar.activation(out=msq[:], in_=mean[:], func=AF.Square)
            nc.vector.scalar_tensor_tensor(
                out=var[:], in0=statg[:, 2:4], scalar=1.0 / n_rows,
                in1=msq[:], op0=ALU.mult, op1=ALU.subtract)
            nc.scalar.activation(out=sd[:], in_=var[:], func=AF.Sqrt, bias=epsc[:])
            nc.vector.reciprocal(out=rstd[:], in_=sd[:])
            nc.vector.tensor_mul(a[:], gc[:], rstd[:])
            nc.vector.tensor_mul(ma[:], mean[:], a[:])
            nc.vector.tensor_sub(z[:], bec[:], ma[:])
            return a, z

        a1, z1 = bn_coeffs(statg1, N1 / (NCORES if NO_CC else 1), g1c, be1c, statp, "bn1_")

        if STAGE <= 1:
            dbg = statp.tile([D, ROWS], F32)
            nc.vector.memset(dbg[:], 0.0)
            nc.vector.tensor_copy(dbg[:, 0:4], stat1[:])
            nc.vector.tensor_copy(dbg[:, 4:6], a1[:])
            nc.vector.tensor_copy(dbg[:, 6:8], z1[:])
            nc.vector.tensor_copy(dbg[:, 8:16], shpart[:])
            nc.vector.tensor_copy(dbg[:, 16:18], crossc[:])
            nc.sync.dma_start(out=outT_d.ap(), in_=dbg[:])
            return nc

        # ------- fold a1; build roundtrip rows: a1rep, z0 rows, c0 ---------
        c0 = statp.tile([D, 2], F32R)
        nc.scalar.activation(out=c0[:], in_=z1[:], func=AF.Prelu, scale=1.0,
                             alpha=SLOPE)
        z1h = statp.tile([D, 2], F32R)
        nc.vector.tensor_scalar_mul(z1h[:], z1[:], 0.5)
        a1r = statp.tile([D, 2], F32R)
        nc.vector.tensor_copy(a1r[:], a1[:])
        a1d = dram.tile([1, F], F32R, tag="a1d", name="a1d")
        nc.sync.dma_start(out=a1d[:].rearrange("x (h p) -> (x p) h", p=D), in_=a1r[:])
        z0hd = dram.tile([1, F], F32R, tag="z0hd", name="z0hd")
        nc.sync.dma_start(out=z0hd[:].rearrange("x (h p) -> (x p) h", p=D), in_=z1h[:])
        c0d = dram.tile([1, F], F32R, tag="c0d", name="c0d")
        nc.sync.dma_start(out=c0d[:].rearrange("x (h p) -> (x p) h", p=D), in_=c0[:])
        a1rep = statp.tile([D, F], F32R)
        nc.sync.dma_start(out=a1rep[:],
                          in_=a1d[:].rearrange("x f -> (x f)").partition_broadcast(D))
        # c0z0 [2, 256]: row0 = c0, row1 = z0/2  (K=2 rank-2 region init lhsT)
        c0z0 = statp.tile([2, F], F32R)
        nc.sync.dma_start(out=c0z0[0:1, :], in_=c0d[:])
        nc.sync.dma_start(out=c0z0[1:2, :], in_=z0hd[:])
        # z04 [1, 4, 256]: z0/2 row replicated 4x for the uv-build span init
        z04 = statp.tile([1, 4, F], F32R)
        for k in range(4):
            nc.sync.dma_start(out=z04[:, k, :], in_=z0hd[:])
        onesrow = statp.tile([1, D], F32R)
        nc.vector.tensor_copy(onesrow[:], onesf[0:1, :])
        w1as = statp.tile([D, F], F32R)
        nc.vector.tensor_mul(w1as[:], w1a[:], a1rep[:])
        w1bs = statp.tile([D, F], F32R)
        nc.vector.tensor_mul(w1bs[:], w1b[:], a1rep[:])

        # ---------------- UVALL build: u'=a1u+z0/2, v'=a1v+z0/2 -------------
        # UVALL [128, 16 q, 256]: partitions 0-31 u(2q), 32-63 v(2q),
        #                         64-95 u(2q+1), 96-127 v(2q+1)
        UVALL = uvp.tile([D, 16, F], F32R, tag="u_rm", name="UVALL")
        for s in range(4):          # spans of 4 q-blocks
            ps = psum.tile([D, 4 * F], F32, tag="span", bufs=2)
            for nh in range(2):
                nc.tensor.matmul(ps[:, nh * 512:(nh + 1) * 512], onesrow[:],
                                 z04[:].rearrange("x k f -> x (k f)")[:, nh * 512:(nh + 1) * 512],
                                 start=True, stop=False, skip_group_check=True)
            for c in range(4):      # q = 4s + c
                q = 4 * s + c
                nc.tensor.matmul(ps[:, c * F:(c + 1) * F], sTu[:, q, :], w1as[:],
                                 start=False, stop=False, skip_group_check=True)
                nc.tensor.matmul(ps[:, c * F:(c + 1) * F], sTv[:, q, :], w1bs[:],
                                 start=False, stop=(c % 2 == 1),
                                 skip_group_check=True)
            if s % 2 == 0:
                nc.scalar.activation(out=UVALL[:, 4 * s:4 * s + 4, :]
                                     .rearrange("p k f -> p (k f)"),
                                     in_=ps[:], func=AF.Copy)
            else:
                nc.vector.tensor_copy(UVALL[:, 4 * s:4 * s + 4, :]
                                      .rearrange("p k f -> p (k f)"), ps[:])

        if STAGE <= 2:
            dbg = statp.tile([D, ROWS], F32)
            nc.vector.memset(dbg[:], 0.0)
            nc.vector.tensor_copy(dbg[:, 0:4 * F],
                                  UVALL[:, 0:4, :].rearrange("p k f -> p (k f)"))
            nc.sync.dma_start(out=outT_d.ap(), in_=dbg[:])
            return nc

        # ---------------- big phase ----------------
        # red regions [128 f-half, 1024 cols], col = 32b + 4ib + c
        red = [psum.tile([D, ROWS], F32, tag=f"red{fh}", bufs=1, name=f"red{fh}")
               for fh in range(2)]
        # region init: rank-2 (c0, z0/2) x (32-deg, 0.01*deg) + statedeg@w1as
        for fh in range(2):
            for nh in range(2):
                nc.tensor.matmul(red[fh][:, nh * 512:(nh + 1) * 512],
                                 c0z0[:, fh * D:(fh + 1) * D],
                                 degA[:, nh * 512:(nh + 1) * 512],
                                 start=True, stop=False, skip_group_check=True)
                nc.tensor.matmul(red[fh][:, nh * 512:(nh + 1) * 512],
                                 w1as[:, fh * D:(fh + 1) * D],
                                 sdeg[:, nh * 512:(nh + 1) * 512],
                                 start=False, stop=False, skip_group_check=True)
        # 0.01 * E @ (a1 v + z0/2): per q-block, cols 64q..64q+64
        for fh in range(2):
            for q in range(16):
                nc.tensor.matmul(red[fh][:, 64 * q:64 * q + 64],
                                 UVALL[:, q, fh * D:(fh + 1) * D],
                                 etbq[:, q, :],
                                 start=False, stop=False, skip_group_check=True)

        # construction + relu + masked reduce, 64 spans of 4 tiles
        for s in range(64):
            ps = psum.tile([D, 4 * F], F32, tag="span", bufs=2)
            for k in range(4):
                t = 4 * s + k
                b, ib = t // 8, t % 8
                nc.tensor.matmul(ps[:, k * F:(k + 1) * F],
                                 selc[:, (b % 2) * 8 + ib, :],
                                 UVALL[:, b // 2, :],
                                 start=True, stop=True)
            m = big.tile([D, 4 * F], F32R, tag="m", bufs=4, name=f"m{s}")
            if DVE_RELU and s % 2 == 1:
                nc.vector.tensor_relu(m[:], ps[:])
            else:
                nc.scalar.activation(out=m[:], in_=ps[:], func=AF.Relu)
            for k in range(4):
                t = 4 * s + k
                b, ib = t // 8, t % 8
                last = t in (127, 255)
                for fh in range(2):
                    nc.tensor.matmul(red[fh][:, 4 * t:4 * t + 4],
                                     m[:, k * F + fh * D:k * F + (fh + 1) * D],
                                     eblk[:, 4 * t:4 * t + 4],
                                     start=False, stop=last,
                                     skip_group_check=True)
        msumT = [big.tile([D, ROWS], F32R, tag=f"msum{fh}", bufs=1,
                          name=f"msum{fh}") for fh in range(2)]
        nc.scalar.activation(out=msumT[0][:], in_=red[0][:], func=AF.Copy)
        nc.vector.tensor_copy(msumT[1][:], red[1][:])

        if STAGE <= 3:
            dbg = statp.tile([D, ROWS], F32)
            nc.vector.tensor_copy(dbg[:], msumT[0][:])
            nc.sync.dma_start(out=outT_d.ap(), in_=dbg[:])
            return nc

        # ---------------- aggT = W2 @ msum + 32*b2 ----------------
        aggT = big.tile([D, ROWS], F32R, tag="aggT", bufs=1, name="aggT")
        for nh in range(2):
            ps = psum.tile([D, 512], F32, tag="span", bufs=2)
            nc.tensor.matmul(ps[:], w2k[:, 0, :], msumT[0][:, nh * 512:(nh + 1) * 512],
                             start=True, stop=False)
            nc.tensor.matmul(ps[:], w2k[:, 1, :], msumT[1][:, nh * 512:(nh + 1) * 512],
                             start=False, stop=True)
            nc.scalar.activation(out=aggT[:, nh * 512:(nh + 1) * 512], in_=ps[:],
                                 func=AF.Identity, bias=b2x32[:], scale=1.0)

        # ---------------- layer 2: H2 = FW1 @ [sT; aggT], stats -------------
        H2 = [big.tile([D, ROWS], F32R, tag=f"h2_{h}", bufs=1, name=f"h2_{h}")
              for h in range(2)]
        st2part = statp.tile([D, 8], F32)
        for fh in range(2):
            for nh in range(2):
                ps = psum.tile([D, 512], F32, tag="span", bufs=2)
                nc.tensor.matmul(ps[:], fw1[:, 0, fh * D:(fh + 1) * D],
                                 sT[:, nh * 512:(nh + 1) * 512], start=True, stop=False)
                nc.tensor.matmul(ps[:], fw1[:, 1, fh * D:(fh + 1) * D],
                                 aggT[:, nh * 512:(nh + 1) * 512], start=False, stop=True)
                c1 = 0 * 4 + fh * 2 + nh
                c2 = 1 * 4 + fh * 2 + nh
                nc.scalar.activation(out=H2[fh][:, nh * 512:(nh + 1) * 512], in_=ps[:],
                                     func=AF.Copy,
                                     accum_out=st2part[:, c1:c1 + 1])
                sq2 = statp.tile([D, 512], F32, tag="sq2")
                nc.scalar.activation(out=sq2[:], in_=H2[fh][:, nh * 512:(nh + 1) * 512],
                                     func=AF.Square,
                                     accum_out=st2part[:, c2:c2 + 1])

        stat2 = statp.tile([D, 4], F32)
        nc.vector.reduce_sum(stat2[:],
                             st2part[:].rearrange("p (s fh nh) -> p (s fh) nh", s=2, nh=2),
                             axis=mybir.AxisListType.X)
        cc2_in = dram.tile([D, 4], F32, tag="cc2i")
        cc2_out = dram.tile([D, 4], F32, tag="cc2o")
        nc.sync.dma_start(out=cc2_in[:], in_=stat2[:])
        if NO_CC:
            nc.sync.dma_start(out=cc2_out[:], in_=cc2_in[:])
        else:
            nc.gpsimd.collective_compute(
                "AllReduce", ALU.add, replica_groups=[list(range(NCORES))],
                ins=[cc2_in[:].opt()], outs=[cc2_out[:].opt()])
        statg2 = statp.tile([D, 4], F32)
        nc.sync.dma_start(out=statg2[:], in_=cc2_out[:])
        a2, z2 = bn_coeffs(statg2, N2 / (NCORES if NO_CC else 1), g2c, be2c, statp, "bn2_")

        # ---------------- m2 = Prelu(a2*H2+z2); outT = FW2 @ m2 + fb2 -------
        m2 = [big.tile([D, ROWS], F32R, tag="m", bufs=4, name=f"m2_{h}")
              for h in range(2)]
        for fh in range(2):
            nc.scalar.activation(out=m2[fh][:], in_=H2[fh][:], func=AF.Prelu,
                                 scale=a2[:, fh:fh + 1], bias=z2[:, fh:fh + 1],
                                 alpha=SLOPE)
        outT = big.tile([D, ROWS], F32, tag="outT", bufs=1, name="outT")
        for nh in range(2):
            ps = psum.tile([D, 512], F32, tag="span", bufs=2)
            nc.tensor.matmul(ps[:], fw2[:, 0, :], m2[0][:, nh * 512:(nh + 1) * 512],
                             start=True, stop=False)
            nc.tensor.matmul(ps[:], fw2[:, 1, :], m2[1][:, nh * 512:(nh + 1) * 512],
                             start=False, stop=True)
            nc.scalar.activation(out=outT[:, nh * 512:(nh + 1) * 512], in_=ps[:],
                                 func=AF.Identity, bias=fb2c[:], scale=1.0)
        nc.sync.dma_start(out=outT_d.ap(), in_=outT[:])
    return nc


def _build_nc_staged():
    nc = _build_nc()
    nc.compile()
    return nc


_NC_CACHE = {}


def _get_nc():
    if "nc" not in _NC_CACHE:
        _NC_CACHE["nc"] = _build_nc_staged()
    return _NC_CACHE["nc"]


def _prep_in_maps(state, edges, msg_w1, msg_b1, msg_gamma, msg_beta, msg_w2,
                  msg_b2, fin_w1, fin_b1, fin_gamma, fin_beta, fin_w2, fin_b2,
                  **_unused):
    f32 = np.float32
    state = np.ascontiguousarray(np.asarray(state, f32))
    edges = np.ascontiguousarray(np.asarray(edges, f32))

    w1aT = np.ascontiguousarray(np.asarray(msg_w1, f32)[:, :D].T)    # [128, 256]
    w1bT = np.ascontiguousarray(np.asarray(msg_w1, f32)[:, D:].T)    # [128, 256]
    w2T = np.ascontiguousarray(np.asarray(msg_w2, f32).T)            # [256, 128]
    fw1T = np.ascontiguousarray(np.asarray(fin_w1, f32).T)           # [256, 256]
    fw2T = np.ascontiguousarray(np.asarray(fin_w2, f32).T)           # [256, 128]

    # selections [128, 16, 128] -> [128, 2048]: idx = (b%2)*8 + ib
    sel = np.zeros((D, 16, D), f32)
    for par in range(2):
        for ib in range(8):
            idx = par * 8 + ib
            for p_ in range(D):
                ii, j = p_ // NOBJ, p_ % NOBJ
                sel[par * 64 + ib * 4 + ii, idx, p_] = 1.0
                sel[par * 64 + NOBJ + j, idx, p_] = 1.0
    sel = sel.reshape(D, 16 * D)

    shared = {
        "w1aT": w1aT, "w1bT": w1bT, "w2T": w2T, "fw1T": fw1T, "fw2T": fw2T,
        "sel_all": np.ascontiguousarray(sel),
        "g1": np.ascontiguousarray(np.asarray(msg_gamma, f32)),
        "be1": np.ascontiguousarray(np.asarray(msg_beta, f32)),
        "b2": np.ascontiguousarray(np.asarray(msg_b2, f32)),
        "g2": np.ascontiguousarray(np.asarray(fin_gamma, f32)),
        "be2": np.ascontiguousarray(np.asarray(fin_beta, f32)),
        "fb2": np.ascontiguousarray(np.asarray(fin_b2, f32)),
    }
    in_maps = []
    for c in range(NCORES):
        sh = state[c * NB:(c + 1) * NB].reshape(ROWS, D)   # [(b,i), d]
        ed = edges[c * NB:(c + 1) * NB].reshape(NB, NOBJ, NOBJ)  # e[b, i, j]
        deg = ed.sum(2)                                    # [b, i]
        cdeg = ed.sum(1)                                   # [b, j]

        # eblk [128, 1024]: col 4t+c, t=(b, ib): 0.99*e[b, ib*4+c, j] at
        # pair rows p = c*32 + j
        eblk = np.zeros((D, ROWS), f32)
        for t in range(NT):
            b, ib = t // 8, t % 8
            for cc in range(4):
                eblk[cc * NOBJ:(cc + 1) * NOBJ, 4 * t + cc] = \
                    (1.0 - SLOPE) * ed[b, ib * 4 + cc, :]

        # etbq [128, 16, 64]: block q: rows 32+j -> cols i (batch 2q),
        # rows 96+j -> cols 32+i (batch 2q+1), value 0.01*e[b, i, j]
        etbq = np.zeros((D, 16, 64), f32)
        for q in range(16):
            etbq[NOBJ:2 * NOBJ, q, 0:NOBJ] = SLOPE * ed[2 * q].T
            etbq[96:128, q, NOBJ:64] = SLOPE * ed[2 * q + 1].T

        degA = np.stack([  # [2, 1024] rows: 32-deg, 0.01*deg  (b,i) order
            (NOBJ - deg).reshape(ROWS), SLOPE * deg.reshape(ROWS)]).astype(f32)
        statedeg = np.ascontiguousarray(sh.T * (SLOPE * deg.reshape(ROWS))[None, :])

        etbd = np.zeros((D, 8, D), f32)      # block-diag E_b^T per 4-batch group
        for g in range(8):
            for bs in range(4):
                etbd[bs * NOBJ:(bs + 1) * NOBJ, g, bs * NOBJ:(bs + 1) * NOBJ] = \
                    ed[g * 4 + bs].T
        degcolp = np.zeros((D, 16), f32)
        degcolp[:, 0::2] = deg.reshape(8, D).T
        cdegcolp = np.zeros((D, 16), f32)
        cdegcolp[:, 0::2] = cdeg.reshape(8, D).T
        sTcols = sh.T.reshape(D, NB, NOBJ)   # [d, b, i]
        sTu = np.zeros((D, 16, 4, NOBJ), f32)
        sTv = np.zeros((D, 16, 4, NOBJ), f32)
        for q in range(16):
            sTu[:, q, 0] = sTcols[:, 2 * q]
            sTu[:, q, 2] = sTcols[:, 2 * q + 1]
            sTv[:, q, 1] = sTcols[:, 2 * q]
            sTv[:, q, 3] = sTcols[:, 2 * q + 1]

        in_maps.append({
            "stateT": np.ascontiguousarray(sh.T),
            "stateTu": np.ascontiguousarray(sTu.reshape(D, 16 * D)),
            "stateTv": np.ascontiguousarray(sTv.reshape(D, 16 * D)),
            "statedeg": statedeg,
            "state_rm": np.ascontiguousarray(sh),
            "etbd_all": np.ascontiguousarray(etbd.reshape(D, 8 * D)),
            "eblk_all": eblk,
            "etbq_all": np.ascontiguousarray(etbq.reshape(D, 16 * 64)),
            "degA": degA,
            "degcol": degcolp, "cdegcol": cdegcolp,
            **shared,
        })
    return in_maps


def kernel(**inputs):
    in_maps = _prep_in_maps(**inputs)
    nc = _get_nc()
    res = run_bass_kernel_spmd(nc, in_maps, core_ids=list(range(NCORES)))
    out = np.empty((B, NOBJ, D), np.float32)
    for c in range(NCORES):
        outT = res.results[c]["outT"]                       # [128, 1024]
        out[c * NB:(c + 1) * NB] = outT.T.reshape(NB, NOBJ, D)
    return out


if __name__ == "__main__":
    print("smoke-building nc...")
    _get_nc()
    print("built OK")
